# revision 1
# baseline (speedup 1.0000x reference)
"""Causal self-attention (B=4, T=2048, C=1024, H=16, D=64) on 8 TRN2 NeuronCores.

Sharding: core = (batch b, head-group g) with b = core // 2, g = core % 2.
Each core computes heads [8g, 8g+8) of batch b and produces the partial
out-projection (C, T) for its head group; the host sums the two head-group
partials per batch and adds the output bias.

On-device layout notes:
- All activations/weights enter the PE as fp16; PSUM accumulates fp32.
- q/k are produced "transposed" (feature on partitions, t on free dim) so
  scores can be computed as ST[s, t] = k^T q with no transposes anywhere.
- RoPE feature permutation per head: rows [e0..e15, o0..o15, e16..e31,
  o16..o31] (e=even/cos-lane of pair i, o=odd). The pair swap is then a
  16-row swap inside each 32-partition quadrant -> one DVE stream_shuffle.
- Softmax runs unnormalized in the (s, t) orientation: E = exp(S/8); the
  per-t denominator is produced by an extra all-ones column appended to V
  (M=65 in the att@V matmul); normalization divides at the end.
- Causal masking: fully-masked (s, t) tiles are skipped; diagonal tiles are
  zeroed elementwise post-exp with gpsimd.affine_select.
"""

import numpy as np

B, T, C = 4, 2048, 1024
H, D = 16, 64
N_CORES = 8
HPG = H // 2            # heads per core (group)
NCHUNK = 4              # head-pair chunks per core
KT = 8                  # k-tiles of 128 over C
KT_AUG = 9              # + bias/ones k-tile
TT = 4                  # t-tiles of 512 over T
NT = 512                # t tile (matmul N)
VS = 66                 # v column stride per head (64 dims + ones + pad)
VW = HPG * VS           # 528 v columns per k-chunk block
ROPE_BASE = 10000.0

_CACHE = {}


def _d_of_r(r):
    # row r (0..63) inside a head's 64 rotated rows -> original head dim d
    f = (r // 32) * 16 + (r % 16)
    return 2 * f + (1 if (r % 32) >= 16 else 0)


def _f_of_p(p):
    # partition p (0..127) -> rope frequency index
    return ((p // 32) % 2) * 16 + (p % 16)


def _build_nc():
    import concourse.bass as bass  # noqa: F401
    import concourse.tile as tile
    from concourse import bacc, mybir
    from contextlib import ExitStack

    f16 = mybir.dt.float16
    f32 = mybir.dt.float32

    nc = bacc.Bacc(
        "TRN2",
        target_bir_lowering=False,
        debug=False,
        enable_asserts=True,
        num_devices=N_CORES,
    )

    xt_d = nc.dram_tensor("xt", (KT_AUG * 128, T), f16, kind="ExternalInput").ap()
    wqk_d = nc.dram_tensor("wqk", (128, KT * 1024), f16, kind="ExternalInput").ap()
    wv_d = nc.dram_tensor("wv", (128, KT_AUG * VW), f16, kind="ExternalInput").ap()
    wo_d = nc.dram_tensor("wo", (128, NCHUNK * 1024), f16, kind="ExternalInput").ap()
    bqk_d = nc.dram_tensor("bqk", (128, 16), f32, kind="ExternalInput").ap()
    cs_d = nc.dram_tensor("cs", (128, T), f16, kind="ExternalInput").ap()
    css_d = nc.dram_tensor("css", (128, T), f16, kind="ExternalInput").ap()
    ot_d = nc.dram_tensor("ot", (1024, T), f32, kind="ExternalOutput").ap()

    SHUF = list(range(16, 32)) + list(range(0, 16))

    with tile.TileContext(nc) as tc:
        with ExitStack() as ctx, nc.allow_low_precision("fp16 attention pipeline"):
            consts = ctx.enter_context(tc.tile_pool(name="consts", bufs=1))
            qk_pool = ctx.enter_context(tc.tile_pool(name="qk", bufs=2))
            rtmp = ctx.enter_context(tc.tile_pool(name="rtmp", bufs=4))
            e_pool = ctx.enter_context(tc.tile_pool(name="e", bufs=8))
            small = ctx.enter_context(tc.tile_pool(name="small", bufs=3))
            osb = ctx.enter_context(tc.tile_pool(name="osb", bufs=6))
            ps_big = ctx.enter_context(tc.tile_pool(name="psbig", bufs=2, space="PSUM"))
            ps_s = ctx.enter_context(tc.tile_pool(name="pss", bufs=2, space="PSUM"))
            ps_y = ctx.enter_context(tc.tile_pool(name="psy", bufs=1, space="PSUM"))

            # ---- resident tiles + input DMA ----
            xt = consts.tile([128, KT_AUG * T], f16)
            for kc in range(KT_AUG):
                for i in range(2):
                    nc.sync.dma_start(xt[:, kc * T + i * 1024: kc * T + (i + 1) * 1024],
                                      xt_d[kc * 128:(kc + 1) * 128, i * 1024:(i + 1) * 1024])
            def dma_split(dst, src, width, parts):
                step = width // parts
                for i in range(parts):
                    nc.sync.dma_start(dst[:, i * step:(i + 1) * step],
                                      src[:, i * step:(i + 1) * step])

            wqk = consts.tile([128, KT * 1024], f16)
            dma_split(wqk, wqk_d, KT * 1024, 8)
            wv = consts.tile([128, KT_AUG * VW], f16)
            dma_split(wv, wv_d, KT_AUG * VW, 4)
            wo = consts.tile([128, NCHUNK * 1024], f16)
            dma_split(wo, wo_d, NCHUNK * 1024, 4)
            bqk = consts.tile([128, 16], f32)
            nc.sync.dma_start(bqk[:], bqk_d[:])
            cs = consts.tile([128, T], f16)
            dma_split(cs, cs_d, T, 2)
            css = consts.tile([128, T], f16)
            dma_split(css, css_d, T, 2)
            v_sb = consts.tile([128, 16 * VW], f16)
            y_all = consts.tile([128, NCHUNK * T], f16)

            # ---- phase 0: V projection for all 8 heads ----
            with nc.named_scope("vproj"):
                for m in range(16):  # 128-row t-slices
                    psa = ps_big.tile([128, 512], f32, tag="big")
                    psb = ps_s.tile([128, 1024], f32, tag="s")
                    for kc in range(KT_AUG):
                        lhs = xt[:, kc * T + m * 128: kc * T + (m + 1) * 128]
                        nc.tensor.matmul(psa[:], lhs, wv[:, kc * VW: kc * VW + 512],
                                         start=(kc == 0), stop=(kc == KT_AUG - 1))
                        nc.tensor.matmul(psb[:, 0:16], lhs, wv[:, kc * VW + 512: (kc + 1) * VW],
                                         start=(kc == 0), stop=(kc == KT_AUG - 1))
                    nc.vector.tensor_copy(v_sb[:, m * VW: m * VW + 512], psa[:])
                    nc.vector.tensor_copy(v_sb[:, m * VW + 512: (m + 1) * VW], psb[:, 0:16])

            for c in range(NCHUNK):
                # ---- phase 1: q/k projection + RoPE for heads (2c, 2c+1) ----
                rq = qk_pool.tile([128, T], f16, tag="rq")
                rk = qk_pool.tile([128, T], f16, tag="rk")
                with nc.named_scope("qkrope"):
                    for tt in range(TT):
                        t0 = tt * NT
                        for which, dst in ((0, rq), (1, rk)):
                            ps = ps_big.tile([128, 512], f32, tag="big")
                            for kc in range(KT):
                                lhsT = wqk[:, kc * 1024 + c * 256 + which * 128: kc * 1024 + c * 256 + which * 128 + 128]
                                rhs = xt[:, kc * T + t0: kc * T + t0 + NT]
                                nc.tensor.matmul(ps[:], lhsT, rhs, start=(kc == 0), stop=(kc == KT - 1))
                            bcol = bqk[:, c * 4 + which * 2: c * 4 + which * 2 + 1]
                            bswp = bqk[:, c * 4 + which * 2 + 1: c * 4 + which * 2 + 2]
                            s_t = rtmp.tile([128, 512], f32, tag="s")
                            nc.vector.stream_shuffle(s_t[:], ps[:], SHUF)
                            x1 = rtmp.tile([128, 512], f16, tag="x1")
                            nc.vector.scalar_tensor_tensor(
                                out=x1[:], in0=ps[:], scalar=bcol, in1=cs[:, t0:t0 + NT],
                                op0=mybir.AluOpType.add, op1=mybir.AluOpType.mult)
                            x2 = rtmp.tile([128, 512], f16, tag="x2")
                            nc.vector.scalar_tensor_tensor(
                                out=x2[:], in0=s_t[:], scalar=bswp, in1=css[:, t0:t0 + NT],
                                op0=mybir.AluOpType.add, op1=mybir.AluOpType.mult)
                            nc.vector.tensor_add(dst[:, t0:t0 + NT], x1[:], x2[:])

                # ---- phase 2: attention for this chunk ----
                # Both heads of the chunk share 1024-wide paired tiles:
                # cols [0:512) = head 2c, [512:1024) = head 2c+1.
                with nc.named_scope("attn"):
                    for tt in range(TT):
                        t0 = tt * NT
                        sc_max = (t0 + NT) // 128
                        yp = ps_y.tile([65, 1024], f32, tag="y")
                        for sc in range(sc_max):
                            s0 = sc * 128
                            dlt = max(0, s0 - t0)  # first causal-valid col in tile
                            w = NT - dlt
                            sp = ps_s.tile([128, 1024], f32, tag="s")
                            nc.tensor.matmul(sp[:, dlt:NT], rk[0:64, s0:s0 + 128],
                                             rq[0:64, t0 + dlt:t0 + NT],
                                             start=True, stop=True, tile_position=(0, 0))
                            nc.tensor.matmul(sp[:, NT + dlt:2 * NT], rk[64:128, s0:s0 + 128],
                                             rq[64:128, t0 + dlt:t0 + NT],
                                             start=True, stop=True, tile_position=(64, 0))
                            e_t = e_pool.tile([128, 1024], f16)
                            s3 = sp[:].rearrange("p (a b) -> p a b", a=2)[:, :, dlt:]
                            e3 = e_t[:].rearrange("p (a b) -> p a b", a=2)[:, :, dlt:]
                            nc.scalar.activation(e3, s3, mybir.ActivationFunctionType.Exp,
                                                 bias=0.0, scale=0.125)
                            if s0 + 127 > t0:
                                # keep iff j' >= p  (j' is offset within the
                                # shrunken width; diagonal starts at col dlt)
                                nc.gpsimd.affine_select(
                                    out=e3, in_=e3,
                                    compare_op=mybir.AluOpType.is_ge,
                                    fill=0.0, base=0,
                                    pattern=[[0, 2], [1, w]], channel_multiplier=-1)
                            for h in range(2):
                                vcol = sc * VW + VS * (2 * c + h)
                                nc.tensor.matmul(yp[:, h * NT + dlt:(h + 1) * NT],
                                                 v_sb[:, vcol: vcol + 65],
                                                 e_t[:, h * NT + dlt:(h + 1) * NT],
                                                 start=(sc == 0), stop=(sc == sc_max - 1),
                                                 skip_group_check=True)
                        # single read of yp frees its PSUM slot immediately;
                        # normalization then runs SBUF-only (2x DVE mode)
                        yc = small.tile([65, 1024], f32, tag="yc")
                        nc.vector.tensor_copy(yc[:], yp[:])
                        rd = small.tile([1, 1024], f32, tag="rd")
                        nc.vector.reciprocal(rd[:], yc[64:65, :])
                        rbc = small.tile([64, 1024], f32, tag="rbc")
                        nc.gpsimd.partition_broadcast(rbc[:], rd[:])
                        for h in range(2):
                            nc.vector.tensor_mul(
                                y_all[h * 64:(h + 1) * 64, c * T + t0: c * T + t0 + NT],
                                yc[0:64, h * NT:(h + 1) * NT],
                                rbc[:, h * NT:(h + 1) * NT])

            # ---- phase 3: output projection (partial over this core's heads) ----
            with nc.named_scope("oproj"):
                for ct in range(8):
                    for tt in range(TT):
                        t0 = tt * NT
                        po = ps_big.tile([128, 512], f32, tag="big")
                        for c in range(NCHUNK):
                            nc.tensor.matmul(po[:], wo[:, c * 1024 + ct * 128: c * 1024 + ct * 128 + 128],
                                             y_all[:, c * T + t0: c * T + t0 + NT],
                                             start=(c == 0), stop=(c == NCHUNK - 1))
                        ob = osb.tile([128, 512], f32)
                        nc.vector.tensor_copy(ob[:], po[:])
                        nc.sync.dma_start(ot_d[ct * 128:(ct + 1) * 128, t0:t0 + NT], ob[:])

    nc.compile()
    return nc


def _prep_inputs(x, qkv_w, qkv_b):
    """Build the 8 per-core input maps (all host-side numpy)."""
    x = np.asarray(x, dtype=np.float32)
    qkv_w = np.asarray(qkv_w, dtype=np.float32)
    qkv_b = np.asarray(qkv_b, dtype=np.float32)

    # xt per batch: (KT_AUG*128, T) fp16 with row 1024 = ones, rest of aug block 0
    xts = []
    for b in range(B):
        xa = np.zeros((KT_AUG * 128, T), dtype=np.float16)
        xa[:C] = x[b].T.astype(np.float16)
        xa[C] = 1.0
        xts.append(xa)

    r = np.arange(64)
    d_r = 2 * ((r // 32) * 16 + (r % 16)) + ((r % 32) >= 16)  # row -> head dim
    p = np.arange(128)
    f_p = ((p // 32) % 2) * 16 + (p % 16)

    ins_g = []
    for g in range(2):
        # wqk: [p, kc*1024 + c*256 + which*128 + m]
        wqk = np.empty((128, KT * 1024), dtype=np.float16)
        bqk = np.empty((128, 16), dtype=np.float32)
        for c in range(NCHUNK):
            for which in range(2):  # 0=q, 1=k
                rows = np.concatenate([
                    which * C + (8 * g + 2 * c + hh) * 64 + d_r for hh in range(2)
                ])  # 128 feature rows
                blk = qkv_w[rows, :]          # (128 feat, 1024 k)
                for kc in range(KT):
                    wqk[:, kc * 1024 + c * 256 + which * 128:
                        kc * 1024 + c * 256 + which * 128 + 128] = \
                        blk[:, kc * 128:(kc + 1) * 128].T.astype(np.float16)
                bc = qkv_b[rows].astype(np.float32)
                bqk[:, c * 4 + which * 2] = bc
                bqk[:, c * 4 + which * 2 + 1] = bc[p ^ 16]
        # wv: [p, kc*VW + col], col = VS*h + j
        wva = np.zeros((KT_AUG * 128, VW), dtype=np.float32)
        for h in range(HPG):
            rows = 2 * C + (8 * g + h) * 64 + np.arange(64)
            wva[:C, VS * h: VS * h + 64] = qkv_w[rows, :].T
            wva[C, VS * h: VS * h + 64] = qkv_b[rows]
            wva[C, VS * h + 64] = 1.0
        wv = np.empty((128, KT_AUG * VW), dtype=np.float16)
        for kc in range(KT_AUG):
            wv[:, kc * VW:(kc + 1) * VW] = wva[kc * 128:(kc + 1) * 128].astype(np.float16)
        ins_g.append((wqk, bqk, wv))

    # rope tables
    inv_freq = (1.0 / (ROPE_BASE ** (np.arange(0, D, 2) / D))).astype(np.float64)
    t = np.arange(T, dtype=np.float64)
    ang = t[None, :] * inv_freq[f_p][:, None]          # (128, T)
    cs = np.cos(ang).astype(np.float16)
    sgn = np.where((p % 32) < 16, -1.0, 1.0)[:, None]
    css = (sgn * np.sin(ang)).astype(np.float16)

    return xts, ins_g, cs, css


def _prep_wo(out_w, g):
    out_w = np.asarray(out_w, dtype=np.float32)
    wo = np.empty((128, NCHUNK * 1024), dtype=np.float16)
    for c in range(NCHUNK):
        rows = np.concatenate([(8 * g + 2 * c + hh) * 64 + np.arange(64) for hh in range(2)])
        wo[:, c * 1024:(c + 1) * 1024] = out_w[:, rows].astype(np.float16).T
    return wo


def kernel(x, qkv_w, qkv_b, out_w, out_b):
    from concourse.bass_utils import run_bass_kernel_spmd

    if "nc" not in _CACHE:
        _CACHE["nc"] = _build_nc()
    nc = _CACHE["nc"]

    xts, ins_g, cs, css = _prep_inputs(x, qkv_w, qkv_b)
    wos = [_prep_wo(out_w, g) for g in range(2)]
    out_b = np.asarray(out_b, dtype=np.float32)

    in_maps = []
    for core in range(N_CORES):
        b, g = core // 2, core % 2
        wqk, bqk, wv = ins_g[g]
        in_maps.append({
            "xt": xts[b], "wqk": wqk, "wv": wv, "wo": wos[g],
            "bqk": bqk, "cs": cs, "css": css,
        })

    try:
        res = run_bass_kernel_spmd(nc, in_maps, core_ids=list(range(N_CORES)))
    except ModuleNotFoundError:
        # BASS_TRACE set but the NTFF profile hook isn't importable here
        import os
        os.environ["BASS_NEVER_TRACE"] = "1"
        res = run_bass_kernel_spmd(nc, in_maps, core_ids=list(range(N_CORES)))

    out = np.empty((B, T, C), dtype=np.float32)
    for b in range(B):
        pt = res.results[2 * b]["ot"] + res.results[2 * b + 1]["ot"]  # (C, T)
        out[b] = pt.T + out_b[None, :]
    return out

